# revision 4
# baseline (speedup 1.0000x reference)
"""Trainium2 Bass kernel for nn_HashEncoder (instant-NGP style hash-grid encoder).

Contract: kernel(inputs, embeddings) -> [1M, 32] f32.
Sharding: data-parallel over points, 8 cores; full 57MB table in each core's HBM.
Device work per core: normalize coords, per level compute corner hash indices on
DVE (uint32 math, 16-bit decomposed multiplies — DVE uint32 mult saturates, but
only the low 19 hash bits are needed), gather 8 corner rows per point via
one batched indirect DMA per (level, col-tile) ([128, 8, cn] offsets),
trilinear-interpolate via broadcast-AP multiplies + segmented tensor_reduce.
Software-pipelined: the interp of level l is emitted after the gather of
level l+1 is issued, so SDMA transfers run back-to-back while DVE computes
the next level's hash indices.
"""
import sys

if "/opt/trn_rl_repo" not in sys.path:
    sys.path.insert(0, "/opt/trn_rl_repo")

import numpy as np

# ---- problem constants (hardcoded per harness contract) ----
D, L, C, H = 3, 16, 2, 16
T = 2 ** 19
BOUND = 1.0
PRIMES = (1, 2654435761, 805459861)
B_FULL = 1_000_000
N_CORES = 8


def _make_offsets():
    offs, o = [0], 0
    for l in range(L):
        res = H * (2 ** l)
        o += min(T, (res + 1) ** D)
        offs.append(o)
    return offs


OFFSETS = _make_offsets()
N_PARAMS = OFFSETS[-1]  # 7131219

# per-core point layout: NPC points = 128 partitions x NC cols, point(p, j) = p*NC + j
NC_COLS = 977
NPC = 128 * NC_COLS          # 125056
B_PAD = NPC * N_CORES        # 1000448
CN_TILE = 192                # cols per SBUF tile
MASK19 = 0x7FFFF


def _build(npc, nc_cols, cn_tile, levels):
    import concourse.bass as bass
    import concourse.tile as tile
    from concourse import bacc, mybir

    dt = mybir.dt
    Alu = mybir.AluOpType
    P = 128

    nc = bacc.Bacc("TRN2", target_bir_lowering=False, debug=False,
                   enable_asserts=False, num_devices=N_CORES,
                   num_swdge_queues=4)
    # round-robin indirect gathers over the 4 SWDGE queues: descriptor
    # generation for queue n runs on Q7 core pair n, parallelizing the
    # ~1us/instruction SWDGE cost that bounds this kernel.
    _qnames = ["qPoolDynamic", "qPoolDynamic1", "qPoolDynamic2",
               "qPoolDynamic3"]
    _qctr = [0]

    def _route(di):
        di.ins.queue = _qnames[_qctr[0] & 3]
        _qctr[0] += 1
        return di
    pts_d = nc.dram_tensor("pts", [npc, 3], dt.float32, kind="ExternalInput")
    emb_d = nc.dram_tensor("emb", [N_PARAMS, C], dt.float32, kind="ExternalInput")
    nout = 2 * len(levels)
    out_d = nc.dram_tensor("out", [npc, nout], dt.float32, kind="ExternalOutput")

    pts_v = pts_d.ap().rearrange("(p n) d -> p n d", p=P)   # [128, nc_cols, 3]
    out_v = out_d.ap().rearrange("(p n) c -> p n c", p=P)   # [128, nc_cols, nout]

    col_tiles = []
    jb = 0
    while jb < nc_cols:
        cn = min(cn_tile, nc_cols - jb)
        col_tiles.append((jb, cn))
        jb += cn

    with tile.TileContext(nc) as tc:
        with tc.tile_pool(name="sb", bufs=2) as sb, \
             tc.tile_pool(name="consts", bufs=1) as cpool:

            # uint32 constant tiles [P, 1] (int immediates aren't supported)
            _consts = {}

            def cu(val):
                if val not in _consts:
                    t = cpool.tile([P, 1], dt.uint32, tag=f"c{val}")
                    nc.vector.memset(t[:, :], val)
                    _consts[val] = t
                return _consts[val][:, :1]

            def ibc(val, shape_free):
                # broadcast [P,1] uint32 const along free dims
                return cu(val).to_broadcast([P] + shape_free)

            def do_interp(pend):
                # trilinear interp of a gathered level: per channel,
                # prod = w8 * feats_c ; segmented reduce over the 8 corners
                li = pend["li"]
                dense = pend["dense"]
                ftile = pend["ftile"]
                w8t = pend["w8t"]
                outt_t = pend["outt"]
                cn = pend["cn"]
                outt_f = outt_t[:, :, :]
                w8_f = w8t[:, :, :]
                for c in range(2):
                    prod = sb.tile([P, cn, 8], dt.float32, tag="prod")
                    prod_f = prod[:, :, :]
                    if dense:
                        fd_f = ftile[:, :, :, :]
                        for bx in range(2):
                            w_v = bass.AP(w8_f.tensor,
                                          w8_f.offset + bx * 4 * cn,
                                          [w8_f.ap[0], [1, cn], [cn, 4]])
                            f_v = bass.AP(fd_f.tensor,
                                          fd_f.offset + bx * 2 + c,
                                          [fd_f.ap[0], [4, cn], [4 * cn, 4]])
                            o_v = bass.AP(prod_f.tensor,
                                          prod_f.offset + bx * 4,
                                          [prod_f.ap[0], [8, cn], [1, 4]])
                            nc.vector.tensor_tensor(
                                out=o_v, in0=w_v, in1=f_v, op=Alu.mult)
                    else:
                        feats_f = ftile[:, :, :, :]
                        w_v = bass.AP(w8_f.tensor, w8_f.offset,
                                      [w8_f.ap[0], [1, cn], [cn, 8]])
                        f_v = bass.AP(feats_f.tensor, feats_f.offset + c,
                                      [feats_f.ap[0], [2, cn], [2 * cn, 8]])
                        nc.vector.tensor_tensor(
                            out=prod_f, in0=w_v, in1=f_v, op=Alu.mult)
                    res_v = bass.AP(outt_f.tensor,
                                    outt_f.offset + li * 2 + c,
                                    [outt_f.ap[0], [nout, cn]])
                    nc.vector.tensor_reduce(
                        out=res_v, in_=prod_f,
                        axis=mybir.AxisListType.X, op=Alu.add)
                if pend["last"]:
                    jb = pend["jb"]
                    nc.sync.dma_start(out=out_v[:, jb:jb + cn, :],
                                      in_=outt_f)

            pending = None

            for ti, (jb, cn) in enumerate(col_tiles):
                pts_t = sb.tile([P, cn, 3], dt.float32, tag="pts")
                nc.sync.dma_start(out=pts_t[:, :, :], in_=pts_v[:, jb:jb + cn, :])

                # xn = clip((pts+1)*0.5, 0, 1)
                xn = sb.tile([P, cn, 3], dt.float32, tag="xn")
                nc.vector.tensor_scalar(
                    out=xn[:, :, :], in0=pts_t[:, :, :], scalar1=0.5, scalar2=0.5,
                    op0=Alu.mult, op1=Alu.add)
                nc.vector.tensor_scalar(
                    out=xn[:, :, :], in0=xn[:, :, :], scalar1=1.0, scalar2=0.0,
                    op0=Alu.min, op1=Alu.max)

                outt = sb.tile([P, cn, nout], dt.float32, tag="outt")

                for li, l in enumerate(levels):
                    res = H * (2 ** l)
                    size = OFFSETS[l + 1] - OFFSETS[l]
                    dense = (res + 1) ** D <= size

                    pos3 = sb.tile([P, cn, 3], dt.float32, tag="pos3")
                    nc.vector.tensor_scalar(
                        out=pos3[:, :, :], in0=xn[:, :, :], scalar1=float(res),
                        scalar2=None, op0=Alu.mult)

                    # floor: r = rint(pos); rf = f32(r); gt = (rf > pos); pgf = rf-gt
                    pgu = sb.tile([P, cn, 3], dt.uint32, tag="pgu")
                    rf = sb.tile([P, cn, 3], dt.float32, tag="rf")
                    gt = sb.tile([P, cn, 3], dt.float32, tag="gtf")
                    nc.vector.tensor_copy(out=pgu[:, :, :], in_=pos3[:, :, :])
                    nc.vector.tensor_copy(out=rf[:, :, :], in_=pgu[:, :, :])
                    nc.vector.tensor_tensor(
                        out=gt[:, :, :], in0=rf[:, :, :], in1=pos3[:, :, :],
                        op=Alu.is_gt)
                    nc.vector.tensor_tensor(
                        out=rf[:, :, :], in0=rf[:, :, :], in1=gt[:, :, :],
                        op=Alu.subtract)
                    nc.vector.tensor_scalar(
                        out=rf[:, :, :], in0=rf[:, :, :], scalar1=float(res - 1),
                        scalar2=None, op0=Alu.min)
                    # f2[0]=1-frac, f2[1]=frac ; frac = pos - pgf
                    f2 = sb.tile([P, 2, 3, cn], dt.float32, tag="f2")
                    frac = sb.tile([P, cn, 3], dt.float32, tag="frac")
                    nc.vector.tensor_tensor(
                        out=frac[:, :, :], in0=pos3[:, :, :], in1=rf[:, :, :],
                        op=Alu.subtract)
                    nc.vector.tensor_copy(out=pgu[:, :, :], in_=rf[:, :, :])
                    for d in range(3):
                        nc.vector.tensor_copy(
                            out=f2[:, 1, d, :], in_=frac[:, :, d])
                        # (frac * -1) - (-1) = 1 - frac
                        nc.vector.tensor_scalar(
                            out=f2[:, 0, d, :], in0=frac[:, :, d], scalar1=-1.0,
                            scalar2=-1.0, op0=Alu.mult, op1=Alu.subtract)

                    # ---- corner term pairs trm[d][0/1]: [P, cn] uint32 ----
                    trm = sb.tile([P, 3, 2, cn], dt.uint32, tag="trm")
                    nc.vector.tensor_copy(out=trm[:, 0, 0, :], in_=pgu[:, :, 0])
                    nc.vector.tensor_tensor(
                        out=trm[:, 0, 1, :], in0=pgu[:, :, 0], in1=ibc(1, [cn]),
                        op=Alu.add)
                    if dense:
                        s1, s2 = res + 1, (res + 1) ** 2
                        for d, s in ((1, s1), (2, s2)):
                            nc.vector.tensor_tensor(
                                out=trm[:, d, 0, :], in0=pgu[:, :, d],
                                in1=ibc(s, [cn]), op=Alu.mult)
                            nc.vector.tensor_tensor(
                                out=trm[:, d, 1, :], in0=trm[:, d, 0, :],
                                in1=ibc(s, [cn]), op=Alu.add)
                    else:
                        # y*p mod 2^19 via 5-bit chunks: products < 2^24 stay
                        # exact through the DVE's float multiply path.
                        nbits = l + 5
                        nch = -(-nbits // 5)
                        for d in (1, 2):
                            p = PRIMES[d]
                            acc = None
                            for jc in range(nch):
                                pk = (p << (5 * jc)) & MASK19
                                nib = sb.tile([P, cn], dt.uint32, tag="nib")
                                if jc == 0:
                                    nc.vector.tensor_tensor(
                                        out=nib[:, :], in0=pgu[:, :, d],
                                        in1=ibc(31, [cn]),
                                        op=Alu.bitwise_and)
                                else:
                                    nc.vector.tensor_tensor(
                                        out=nib[:, :], in0=pgu[:, :, d],
                                        in1=ibc(5 * jc, [cn]),
                                        op=Alu.logical_shift_right)
                                    nc.vector.tensor_tensor(
                                        out=nib[:, :], in0=nib[:, :],
                                        in1=ibc(31, [cn]),
                                        op=Alu.bitwise_and)
                                nc.vector.tensor_tensor(
                                    out=nib[:, :], in0=nib[:, :],
                                    in1=ibc(pk, [cn]), op=Alu.mult)
                                nc.vector.tensor_tensor(
                                    out=nib[:, :], in0=nib[:, :],
                                    in1=ibc(MASK19, [cn]),
                                    op=Alu.bitwise_and)
                                if acc is None:
                                    acc = sb.tile([P, cn], dt.uint32, tag="hacc")
                                    nc.vector.tensor_copy(out=acc[:, :], in_=nib[:, :])
                                else:
                                    nc.vector.tensor_tensor(
                                        out=acc[:, :], in0=acc[:, :],
                                        in1=nib[:, :], op=Alu.add)
                            nc.vector.tensor_copy(out=trm[:, d, 0, :], in_=acc[:, :])
                            nc.vector.tensor_tensor(
                                out=trm[:, d, 1, :], in0=trm[:, d, 0, :],
                                in1=ibc(p & MASK19, [cn]), op=Alu.add)

                    # ---- combine to 8 corner indices (k = bx*4 + by*2 + bz) ----
                    comb_op = Alu.add if dense else Alu.bitwise_xor
                    trm_f = trm[:, :, :, :]
                    part = trm_f.ap[0]
                    xy = sb.tile([P, 2, 2, cn], dt.uint32, tag="xy")
                    in_x = bass.AP(trm_f.tensor, trm[:, 0, 0, :].offset,
                                   [part, [cn, 2], [0, 2], [1, cn]])
                    in_y = bass.AP(trm_f.tensor, trm[:, 1, 0, :].offset,
                                   [part, [0, 2], [cn, 2], [1, cn]])
                    nc.vector.tensor_tensor(
                        out=xy[:, :, :, :], in0=in_x, in1=in_y, op=comb_op)
                    idx8 = sb.tile([P, 8, cn], dt.uint32, tag="idx8")
                    xy_f = xy[:, :, :, :]
                    idx8_f = idx8[:, :, :]
                    for bz in range(2):
                        in_xy = bass.AP(xy_f.tensor, xy_f.offset,
                                        [xy_f.ap[0], [2 * cn, 2], [cn, 2],
                                         [1, cn]])
                        in_z = bass.AP(trm_f.tensor,
                                       trm[:, 2, bz, :].offset,
                                       [part, [0, 2], [0, 2], [1, cn]])
                        o_z = bass.AP(idx8_f.tensor, idx8_f.offset + bz * cn,
                                      [idx8_f.ap[0], [4 * cn, 2], [2 * cn, 2],
                                       [1, cn]])
                        nc.vector.tensor_tensor(
                            out=o_z, in0=in_xy, in1=in_z, op=comb_op)
                    if not dense:
                        nc.vector.tensor_tensor(
                            out=idx8[:, :, :], in0=idx8[:, :, :],
                            in1=ibc(MASK19, [8, cn]),
                            op=Alu.bitwise_and)
                    nc.vector.tensor_tensor(
                        out=idx8[:, :, :], in0=idx8[:, :, :],
                        in1=ibc(OFFSETS[l], [8, cn]), op=Alu.add)

                    # ---- gathers: [128,1]-offset indirect DMAs (HW contract:
                    # one descriptor per partition per instruction)
                    idx8_i = idx8[:, :, :].bitcast(dt.int32)
                    if dense:
                        # corners (x, x+1) are consecutive rows: fetch both
                        # (16B) per offset. Layout [P, yz, cn, (x, c)].
                        ftile = sb.tile([P, 4, cn, 4], dt.float32,
                                        tag="featsd")
                        for k in range(4):
                            for j in range(cn):
                                _route(nc.gpsimd.indirect_dma_start(
                                    out=ftile[:, k, j, :],
                                    out_offset=None,
                                    in_=emb_d[:, :],
                                    in_offset=bass.IndirectOffsetOnAxis(
                                        ap=idx8_i[:, k, j:j + 1], axis=0),
                                ))
                    else:
                        ftile = sb.tile([P, 8, cn, 2], dt.float32, tag="feats")
                        for k in range(8):
                            for j in range(cn):
                                _route(nc.gpsimd.indirect_dma_start(
                                    out=ftile[:, k, j, :],
                                    out_offset=None,
                                    in_=emb_d[:, :],
                                    in_offset=bass.IndirectOffsetOnAxis(
                                        ap=idx8_i[:, k, j:j + 1], axis=0),
                                ))

                    # ---- weights: w8[k] = fx_bx * fy_by * fz_bz ----
                    f2_f = f2[:, :, :, :]
                    xyw = sb.tile([P, 2, 2, cn], dt.float32, tag="xyw")
                    wx = bass.AP(f2_f.tensor, f2[:, 0, 0, :].offset,
                                 [f2_f.ap[0], [3 * cn, 2], [0, 2], [1, cn]])
                    wy = bass.AP(f2_f.tensor, f2[:, 0, 1, :].offset,
                                 [f2_f.ap[0], [0, 2], [3 * cn, 2], [1, cn]])
                    nc.vector.tensor_tensor(
                        out=xyw[:, :, :, :], in0=wx, in1=wy, op=Alu.mult)
                    w8t = sb.tile([P, 8, cn], dt.float32, tag="w8")
                    xyw_f = xyw[:, :, :, :]
                    w8_f = w8t[:, :, :]
                    for bz in range(2):
                        in_xyw = bass.AP(xyw_f.tensor, xyw_f.offset,
                                         [xyw_f.ap[0], [2 * cn, 2], [cn, 2],
                                          [1, cn]])
                        wz = bass.AP(f2_f.tensor,
                                     f2[:, bz, 2, :].offset,
                                     [f2_f.ap[0], [0, 2], [0, 2], [1, cn]])
                        o_w = bass.AP(w8_f.tensor, w8_f.offset + bz * cn,
                                      [w8_f.ap[0], [4 * cn, 2], [2 * cn, 2],
                                       [1, cn]])
                        nc.vector.tensor_tensor(out=o_w, in0=in_xyw, in1=wz,
                                                op=Alu.mult)

                    # ---- flush previous level's interp (its gather has had a
                    # full level of compute to complete; ours is now in flight)
                    if pending is not None:
                        do_interp(pending)
                    pending = dict(li=li, dense=dense, ftile=ftile, w8t=w8t,
                                   outt=outt, cn=cn, jb=jb,
                                   last=(li == len(levels) - 1))

            do_interp(pending)

    nc.compile()
    return nc


_BUILD_CACHE = {}


def _get_nc(npc, nc_cols, cn_tile, levels):
    key = (npc, nc_cols, cn_tile, tuple(levels))
    if key not in _BUILD_CACHE:
        _BUILD_CACHE[key] = _build(npc, nc_cols, cn_tile, levels)
    return _BUILD_CACHE[key]


def kernel(inputs: np.ndarray, embeddings: np.ndarray, _trace=False) -> np.ndarray:
    from concourse.bass_utils import run_bass_kernel_spmd

    inputs = np.ascontiguousarray(inputs, dtype=np.float32)
    embeddings = np.ascontiguousarray(embeddings, dtype=np.float32)
    B = inputs.shape[0]

    pts_pad = np.zeros((B_PAD, 3), dtype=np.float32)
    pts_pad[:B] = inputs
    nc = _get_nc(NPC, NC_COLS, CN_TILE, list(range(L)))
    in_maps = [dict(pts=pts_pad[c * NPC:(c + 1) * NPC], emb=embeddings)
               for c in range(N_CORES)]
    import time as _time
    _t0 = _time.time()
    r = run_bass_kernel_spmd(nc, in_maps, core_ids=list(range(N_CORES)),
                             trace=_trace)
    kernel._last_result = r
    kernel._last_wall_s = _time.time() - _t0
    out = np.concatenate([r.results[c]["out"] for c in range(N_CORES)], axis=0)
    kernel._last_exec_ns = r.exec_time_ns
    return out[:B]



# revision 5
# speedup vs baseline: 1.4091x; 1.4091x over previous
"""Trainium2 Bass kernel for nn_HashEncoder (instant-NGP hash-grid encoder).

Contract: kernel(inputs, embeddings) -> [1M, 32] f32.
Sharding: data-parallel over points, 8 cores; full table in each core's HBM.

Gather strategy: the per-instruction indirect-DMA path costs ~1.15us per 128
descriptors (Pool-seq/SWDGE serial), so all corner fetches go through
InstDMAGatherAnt (dma_gather), which amortizes thousands of descriptors per
instruction and parallelizes descriptor generation across the 4 SWDGE queues.
dma_gather addresses 256B blocks via int16 indices, so a preprocessing pass
builds, per level, two copies of the level table (row-shifted by 0 and 16),
making every row reachable at in-block position idx&15 of block
(idx>>5) + ((idx>>4)&1)*nblk  (max 2*16384 = 2^15 blocks, exactly int16).
The 8B row is extracted from each gathered 256B block on DVE via a one-hot
mask (is_equal vs iota16) + multiply + segmented reduce; dense levels extract
the two consecutive rows (x, x+1) from the same block with the same mask.
"""
import sys

if "/opt/trn_rl_repo" not in sys.path:
    sys.path.insert(0, "/opt/trn_rl_repo")

import numpy as np

# ---- problem constants (hardcoded per harness contract) ----
D, L, C, H = 3, 16, 2, 16
T = 2 ** 19
BOUND = 1.0
PRIMES = (1, 2654435761, 805459861)
B_FULL = 1_000_000
N_CORES = 8


def _make_offsets():
    offs, o = [0], 0
    for l in range(L):
        res = H * (2 ** l)
        o += min(T, (res + 1) ** D)
        offs.append(o)
    return offs


OFFSETS = _make_offsets()
N_PARAMS = OFFSETS[-1]  # 7131219

# per-core point layout: NPC points = 128 partitions x NC cols, point(p, j) = p*NC + j
NC_COLS = 977
NPC = 128 * NC_COLS          # 125056
B_PAD = NPC * N_CORES        # 1000448
CN_TILE = 96                 # cols per SBUF tile
MASK19 = 0x7FFFF
NIDX = 4096                  # gather requests per dma_gather instruction
GCOLS = NIDX // 128          # 48 gathered blocks per partition per instr

# per-level block-table geometry (2 shifted copies of 32-row blocks)
def _lvl_geom():
    g = []
    base = 0  # in 32-row blocks within tbl2
    for l in range(L):
        size = OFFSETS[l + 1] - OFFSETS[l]
        nblk = -(-size // 32)
        nblk += nblk & 1          # even so copies stay 128-f32 aligned
        g.append((size, nblk, base))
        base += 2 * nblk
    return g, base


LGEOM, TBL2_BLOCKS = _lvl_geom()


def _build(npc, nc_cols, cn_tile, levels):
    import concourse.bass as bass
    import concourse.tile as tile
    from concourse import bacc, mybir

    dt = mybir.dt
    Alu = mybir.AluOpType
    P = 128

    nc = bacc.Bacc("TRN2", target_bir_lowering=False, debug=False,
                   enable_asserts=False, num_devices=N_CORES,
                   num_swdge_queues=4)
    pts_d = nc.dram_tensor("pts", [npc, 3], dt.float32, kind="ExternalInput")
    emb_d = nc.dram_tensor("emb", [N_PARAMS, C], dt.float32, kind="ExternalInput")
    nout = 2 * len(levels)
    out_d = nc.dram_tensor("out", [npc, nout], dt.float32, kind="ExternalOutput")
    # block table: per level 2 shifted copies, 32 rows (64 f32) per block
    # (+8 blocks of slack for zero-fill overshoot)
    tbl2_d = nc.dram_tensor("tbl2", [(TBL2_BLOCKS + 8) * 32, 2], dt.float32,
                            kind="Internal")
    # int16 scratch for the idx partition-fold roundtrip (4 rotating slots)
    scr_d = nc.dram_tensor("scri", [4 * 8 * cn_tile * 128], dt.int16,
                           kind="Internal")

    pts_v = pts_d.ap().rearrange("(p n) d -> p n d", p=P)   # [128, nc_cols, 3]
    out_v = out_d.ap().rearrange("(p n) c -> p n c", p=P)   # [128, nc_cols, nout]
    emb_flat = emb_d.ap().rearrange("(p n) c -> p (n c)", p=1)  # [1, N*2] view
    tbl2_flat = tbl2_d.ap().rearrange("(p n) c -> p (n c)", p=1)
    scr_v = scr_d.ap()

    col_tiles = []
    jb = 0
    while jb < nc_cols:
        cn = min(cn_tile, nc_cols - jb)
        col_tiles.append((jb, cn))
        jb += cn

    _qctr = [0]

    def next_q():
        q = _qctr[0] & 3
        _qctr[0] += 1
        return q

    with tile.TileContext(nc) as tc:
        with tc.tile_pool(name="sb", bufs=2) as sb, \
             tc.tile_pool(name="consts", bufs=1) as cpool:

            _consts = {}

            def cu(val):
                if val not in _consts:
                    t = cpool.tile([P, 1], dt.uint32, tag=f"c{val}")
                    nc.vector.memset(t[:, :], val)
                    _consts[val] = t
                return _consts[val][:, :1]

            def ibc(val, shape_free):
                return cu(val).to_broadcast([P] + shape_free)

            # ---- preprocessing: build tbl2 (2 shifted copies per level) ----
            # bounce through SBUF in [128, 4096] f32 (2MB) chunks
            CHW = 1024
            zt = cpool.tile([P, 512], dt.float32, tag="zt")
            nc.vector.memset(zt[:, :], 0.0)
            for l in range(L):
                size, nblk, base = LGEOM[l]
                for cpy in range(2):
                    src0 = (OFFSETS[l] + 16 * cpy) * 2        # f32 offset
                    dst0 = (base + cpy * nblk) * 64
                    want = nblk * 32 * 2                       # f32 to fill
                    avail = max(0, min(want, N_PARAMS * 2 - src0))
                    o = 0
                    while o + P <= avail:
                        w = min(CHW, (avail - o) // P)
                        bt = sb.tile([P, CHW], dt.float32, tag="ppb")
                        nc.sync.dma_start(
                            out=bt[:, :w],
                            in_=bass.AP(emb_flat.tensor, src0 + o,
                                        [[w, P], [1, w]]))
                        nc.sync.dma_start(
                            out=bass.AP(tbl2_flat.tensor, dst0 + o,
                                        [[w, P], [1, w]]),
                            in_=bt[:, :w])
                        o += P * w
                    rem = avail - o
                    if rem > 0:          # <128 f32 tail of real data
                        bt = sb.tile([P, CHW], dt.float32, tag="ppb")
                        nc.sync.dma_start(
                            out=bt[0:1, :rem],
                            in_=bass.AP(emb_flat.tensor, src0 + o,
                                        [[rem, 1], [1, rem]]))
                        nc.sync.dma_start(
                            out=bass.AP(tbl2_flat.tensor, dst0 + o,
                                        [[rem, 1], [1, rem]]),
                            in_=bt[0:1, :rem])
                        o += rem
                    # zero the remainder [o, want) (NaN safety; may overshoot
                    # by <512 f32 into the next region, written later / slack)
                    z = want - o
                    if z > 0:
                        zr = min(P, z)
                        nw = -(-z // zr)
                        nc.sync.dma_start(
                            out=bass.AP(tbl2_flat.tensor, dst0 + o,
                                        [[nw, zr], [1, nw]]),
                            in_=zt[:zr, :nw])

            # iota16 f32 constant [P, 16]
            iota16 = cpool.tile([P, 16], dt.float32, tag="iota16")
            iotau = cpool.tile([P, 16], dt.uint32, tag="iotau")
            nc.gpsimd.iota(iotau[:, :], pattern=[[1, 16]], base=0,
                           channel_multiplier=0)
            nc.vector.tensor_copy(out=iota16[:, :], in_=iotau[:, :])

            def do_interp(pend):
                # trilinear interp: per channel prod = w * feats, segmented
                # reduce over corners (identical memory layout to v1)
                li = pend["li"]
                dense = pend["dense"]
                feats_t = pend["feats"]
                w8t = pend["w8t"]
                outt_t = pend["outt"]
                cn = pend["cn"]
                outt_f = outt_t[:, :, :]
                w8_f = w8t[:, :, :]
                feats_f = feats_t[:, :, :]
                for c in range(2):
                    prod = sb.tile([P, cn, 8], dt.float32, tag="prod")
                    prod_f = prod[:, :, :]
                    if dense:
                        for bx in range(2):
                            w_v = bass.AP(w8_f.tensor,
                                          w8_f.offset + bx * 4 * cn,
                                          [w8_f.ap[0], [1, cn], [cn, 4]])
                            f_v = bass.AP(feats_f.tensor,
                                          feats_f.offset + bx * 2 + c,
                                          [feats_f.ap[0], [4, cn], [4 * cn, 4]])
                            o_v = bass.AP(prod_f.tensor,
                                          prod_f.offset + bx * 4,
                                          [prod_f.ap[0], [8, cn], [1, 4]])
                            nc.vector.tensor_tensor(
                                out=o_v, in0=w_v, in1=f_v, op=Alu.mult)
                    else:
                        w_v = bass.AP(w8_f.tensor, w8_f.offset,
                                      [w8_f.ap[0], [1, cn], [cn, 8]])
                        f_v = bass.AP(feats_f.tensor, feats_f.offset + c,
                                      [feats_f.ap[0], [2, cn], [2 * cn, 8]])
                        nc.vector.tensor_tensor(
                            out=prod_f, in0=w_v, in1=f_v, op=Alu.mult)
                    res_v = bass.AP(outt_f.tensor,
                                    outt_f.offset + li * 2 + c,
                                    [outt_f.ap[0], [nout, cn]])
                    nc.vector.tensor_reduce(
                        out=res_v, in_=prod_f,
                        axis=mybir.AxisListType.X, op=Alu.add)
                if pend["last"]:
                    jb = pend["jb"]
                    nc.sync.dma_start(out=out_v[:, jb:jb + cn, :],
                                      in_=outt_f)

            pending = None

            for ti, (jb, cn) in enumerate(col_tiles):
                pts_t = sb.tile([P, cn, 3], dt.float32, tag="pts")
                nc.sync.dma_start(out=pts_t[:, :, :], in_=pts_v[:, jb:jb + cn, :])

                xn = sb.tile([P, cn, 3], dt.float32, tag="xn")
                nc.vector.tensor_scalar(
                    out=xn[:, :, :], in0=pts_t[:, :, :], scalar1=0.5, scalar2=0.5,
                    op0=Alu.mult, op1=Alu.add)
                nc.vector.tensor_scalar(
                    out=xn[:, :, :], in0=xn[:, :, :], scalar1=1.0, scalar2=0.0,
                    op0=Alu.min, op1=Alu.max)

                outt = sb.tile([P, cn, nout], dt.float32, tag="outt")

                for li, l in enumerate(levels):
                    res = H * (2 ** l)
                    size, nblk, lbase = LGEOM[l]
                    dense = (res + 1) ** D <= size
                    K = 4 if dense else 8

                    pos3 = sb.tile([P, cn, 3], dt.float32, tag="pos3")
                    nc.vector.tensor_scalar(
                        out=pos3[:, :, :], in0=xn[:, :, :], scalar1=float(res),
                        scalar2=None, op0=Alu.mult)

                    pgu = sb.tile([P, cn, 3], dt.uint32, tag="pgu")
                    rf = sb.tile([P, cn, 3], dt.float32, tag="rf")
                    gt = sb.tile([P, cn, 3], dt.float32, tag="gtf")
                    nc.vector.tensor_copy(out=pgu[:, :, :], in_=pos3[:, :, :])
                    nc.vector.tensor_copy(out=rf[:, :, :], in_=pgu[:, :, :])
                    nc.vector.tensor_tensor(
                        out=gt[:, :, :], in0=rf[:, :, :], in1=pos3[:, :, :],
                        op=Alu.is_gt)
                    nc.vector.tensor_tensor(
                        out=rf[:, :, :], in0=rf[:, :, :], in1=gt[:, :, :],
                        op=Alu.subtract)
                    nc.vector.tensor_scalar(
                        out=rf[:, :, :], in0=rf[:, :, :], scalar1=float(res - 1),
                        scalar2=None, op0=Alu.min)
                    f2 = sb.tile([P, 2, 3, cn], dt.float32, tag="f2")
                    frac = sb.tile([P, cn, 3], dt.float32, tag="frac")
                    nc.vector.tensor_tensor(
                        out=frac[:, :, :], in0=pos3[:, :, :], in1=rf[:, :, :],
                        op=Alu.subtract)
                    nc.vector.tensor_copy(out=pgu[:, :, :], in_=rf[:, :, :])
                    for d in range(3):
                        nc.vector.tensor_copy(
                            out=f2[:, 1, d, :], in_=frac[:, :, d])
                        nc.vector.tensor_scalar(
                            out=f2[:, 0, d, :], in0=frac[:, :, d], scalar1=-1.0,
                            scalar2=-1.0, op0=Alu.mult, op1=Alu.subtract)

                    # ---- corner term pairs trm[d][0/1]: [P, cn] uint32 ----
                    trm = sb.tile([P, 3, 2, cn], dt.uint32, tag="trm")
                    nc.vector.tensor_copy(out=trm[:, 0, 0, :], in_=pgu[:, :, 0])
                    nc.vector.tensor_tensor(
                        out=trm[:, 0, 1, :], in0=pgu[:, :, 0], in1=ibc(1, [cn]),
                        op=Alu.add)
                    if dense:
                        s1, s2 = res + 1, (res + 1) ** 2
                        for d, s in ((1, s1), (2, s2)):
                            nc.vector.tensor_tensor(
                                out=trm[:, d, 0, :], in0=pgu[:, :, d],
                                in1=ibc(s, [cn]), op=Alu.mult)
                            nc.vector.tensor_tensor(
                                out=trm[:, d, 1, :], in0=trm[:, d, 0, :],
                                in1=ibc(s, [cn]), op=Alu.add)
                    else:
                        nbits = l + 5
                        nch = -(-nbits // 5)
                        for d in (1, 2):
                            p = PRIMES[d]
                            acc = None
                            for jc in range(nch):
                                pk = (p << (5 * jc)) & MASK19
                                nib = sb.tile([P, cn], dt.uint32, tag="nib")
                                if jc == 0:
                                    nc.vector.tensor_tensor(
                                        out=nib[:, :], in0=pgu[:, :, d],
                                        in1=ibc(31, [cn]),
                                        op=Alu.bitwise_and)
                                else:
                                    nc.vector.tensor_tensor(
                                        out=nib[:, :], in0=pgu[:, :, d],
                                        in1=ibc(5 * jc, [cn]),
                                        op=Alu.logical_shift_right)
                                    nc.vector.tensor_tensor(
                                        out=nib[:, :], in0=nib[:, :],
                                        in1=ibc(31, [cn]),
                                        op=Alu.bitwise_and)
                                nc.vector.tensor_tensor(
                                    out=nib[:, :], in0=nib[:, :],
                                    in1=ibc(pk, [cn]), op=Alu.mult)
                                nc.vector.tensor_tensor(
                                    out=nib[:, :], in0=nib[:, :],
                                    in1=ibc(MASK19, [cn]),
                                    op=Alu.bitwise_and)
                                if acc is None:
                                    acc = sb.tile([P, cn], dt.uint32, tag="hacc")
                                    nc.vector.tensor_copy(out=acc[:, :], in_=nib[:, :])
                                else:
                                    nc.vector.tensor_tensor(
                                        out=acc[:, :], in0=acc[:, :],
                                        in1=nib[:, :], op=Alu.add)
                            nc.vector.tensor_copy(out=trm[:, d, 0, :], in_=acc[:, :])
                            nc.vector.tensor_tensor(
                                out=trm[:, d, 1, :], in0=trm[:, d, 0, :],
                                in1=ibc(p & MASK19, [cn]), op=Alu.add)

                    # ---- combine to K corner indices (k = bx*4 + by*2 + bz) ----
                    comb_op = Alu.add if dense else Alu.bitwise_xor
                    trm_f = trm[:, :, :, :]
                    part = trm_f.ap[0]
                    xy = sb.tile([P, 2, 2, cn], dt.uint32, tag="xy")
                    in_x = bass.AP(trm_f.tensor, trm[:, 0, 0, :].offset,
                                   [part, [cn, 2], [0, 2], [1, cn]])
                    in_y = bass.AP(trm_f.tensor, trm[:, 1, 0, :].offset,
                                   [part, [0, 2], [cn, 2], [1, cn]])
                    nc.vector.tensor_tensor(
                        out=xy[:, :, :, :], in0=in_x, in1=in_y, op=comb_op)
                    idx8 = sb.tile([P, 8, cn], dt.uint32, tag="idx8")
                    xy_f = xy[:, :, :, :]
                    idx8_f = idx8[:, :, :]
                    for bz in range(2):
                        in_xy = bass.AP(xy_f.tensor, xy_f.offset,
                                        [xy_f.ap[0], [2 * cn, 2], [cn, 2],
                                         [1, cn]])
                        in_z = bass.AP(trm_f.tensor,
                                       trm[:, 2, bz, :].offset,
                                       [part, [0, 2], [0, 2], [1, cn]])
                        o_z = bass.AP(idx8_f.tensor, idx8_f.offset + bz * cn,
                                      [idx8_f.ap[0], [4 * cn, 2], [2 * cn, 2],
                                       [1, cn]])
                        nc.vector.tensor_tensor(
                            out=o_z, in0=in_xy, in1=in_z, op=comb_op)
                    if not dense:
                        nc.vector.tensor_tensor(
                            out=idx8[:, :, :], in0=idx8[:, :, :],
                            in1=ibc(MASK19, [8, cn]),
                            op=Alu.bitwise_and)

                    # ---- block index + in-block position (level-relative) ----
                    kcn = K * cn
                    idxK = bass.AP(idx8_f.tensor, idx8_f.offset,
                                   [idx8_f.ap[0], [1, kcn]])
                    bi = sb.tile([P, kcn], dt.uint32, tag="bi")
                    tmp = sb.tile([P, kcn], dt.uint32, tag="btmp")
                    nc.vector.tensor_tensor(
                        out=bi[:, :kcn], in0=idxK, in1=ibc(5, [kcn]),
                        op=Alu.logical_shift_right)
                    nc.vector.tensor_tensor(
                        out=tmp[:, :kcn], in0=idxK, in1=ibc(4, [kcn]),
                        op=Alu.logical_shift_right)
                    nc.vector.tensor_tensor(
                        out=tmp[:, :kcn], in0=tmp[:, :kcn], in1=ibc(1, [kcn]),
                        op=Alu.bitwise_and)
                    nc.vector.tensor_tensor(
                        out=tmp[:, :kcn], in0=tmp[:, :kcn],
                        in1=ibc(nblk, [kcn]), op=Alu.mult)
                    nc.vector.tensor_tensor(
                        out=bi[:, :kcn], in0=bi[:, :kcn], in1=tmp[:, :kcn],
                        op=Alu.add)
                    posf = sb.tile([P, kcn], dt.float32, tag="posf")
                    nc.vector.tensor_tensor(
                        out=tmp[:, :kcn], in0=idxK, in1=ibc(15, [kcn]),
                        op=Alu.bitwise_and)
                    nc.vector.tensor_copy(out=posf[:, :kcn], in_=tmp[:, :kcn])
                    bi16 = sb.tile([P, kcn], dt.int16, tag="bi16")
                    nc.vector.tensor_copy(out=bi16[:, :kcn], in_=bi[:, :kcn])

                    # ---- wrap to dma_gather idx layout ----
                    # request i = c*128 + (16g+q) needs its block index at
                    # idxs[q, 8c+g]. Fold partitions via a DRAM roundtrip,
                    # interleave (g,c)->(c,g) on DVE, replicate to all groups.
                    scr_s = (li % 4) * (8 * cn * P)      # int16 slot
                    nc.sync.dma_start(
                        out=bass.AP(scr_v.tensor, scr_s,
                                    [[kcn, P], [1, kcn]]),
                        in_=bi16[:, :kcn])
                    nreq = kcn * P
                    nchunk = -(-nreq // NIDX)
                    ncols16 = nchunk * (NIDX // 16)
                    wrapT = sb.tile([P, 8 * kcn], dt.int16, tag="wrapT")
                    # wrapT[q, g*kcn + c] = scr[(16g+q)*kcn + c]
                    nc.sync.dma_start(
                        out=wrapT[0:16, :],
                        in_=bass.AP(scr_v.tensor, scr_s,
                                    [[kcn, 16], [16 * kcn, 8], [1, kcn]]))
                    idxw = sb.tile([P, ncols16], dt.int16, tag="idxw")
                    if ncols16 > 8 * kcn:
                        nc.vector.memset(idxw[:, :], 0)
                    wf = wrapT[:, :]
                    of = idxw[:, :]
                    nc.vector.tensor_copy(
                        out=bass.AP(of.tensor, of.offset,
                                    [[of.ap[0][0], 16], [8, kcn], [1, 8]]),
                        in_=bass.AP(wf.tensor, wf.offset,
                                    [[wf.ap[0][0], 16], [1, kcn], [kcn, 8]]))
                    # replicate partitions 0:16 -> all 8 groups
                    for g in range(1, 8):
                        nc.sync.dma_start(
                            out=idxw[16 * g:16 * (g + 1), :],
                            in_=idxw[0:16, :])

                    # ---- gathers: dma_gather per chunk of NIDX requests ----
                    in_ap = bass.AP(tbl2_flat.tensor, lbase * 64,
                                    [[64, 2 * nblk], [1, 64]])
                    feats = sb.tile([P, kcn, 2 if not dense else 4],
                                    dt.float32,
                                    tag="featsd" if dense else "feats")
                    gts = []
                    for ci in range(nchunk):
                        gtile = sb.tile([P, GCOLS, 64], dt.float32,
                                        tag="gt")
                        nc.gpsimd.dma_gather(
                            out_ap=gtile[:, :, :], in_ap=in_ap,
                            idxs_ap=idxw[:, ci * (NIDX // 16):
                                         (ci + 1) * (NIDX // 16)],
                            num_idxs=NIDX, num_idxs_reg=NIDX,
                            elem_size=64, queue_num=next_q(),
                            single_packet=False)
                        gts.append((ci, gtile))

                    # ---- extraction: one-hot over 16 rows ----
                    nval = 2 if not dense else 4
                    feats_f = feats[:, :, :]
                    for ci, gtile in gts:
                        c0 = ci * GCOLS            # corner-col base
                        ccols = min(GCOLS, kcn - c0)
                        if ccols <= 0:
                            break
                        gf = gtile[:, :, :]
                        mask = sb.tile([P, GCOLS, 16], dt.float32, tag="mask")
                        mf = mask[:, :, :]
                        pf = posf[:, :]
                        nc.vector.tensor_tensor(
                            out=bass.AP(mf.tensor, mf.offset,
                                        [mf.ap[0], [16, ccols], [1, 16]]),
                            in0=bass.AP(iota16[:, :].tensor,
                                        iota16[:, :].offset,
                                        [iota16[:, :].ap[0], [0, ccols],
                                         [1, 16]]),
                            in1=bass.AP(pf.tensor, pf.offset + c0,
                                        [pf.ap[0], [1, ccols], [0, 16]]),
                            op=Alu.is_equal)
                        junk = sb.tile([P, GCOLS, 16], dt.float32, tag="junk")
                        jf = junk[:, :, :]
                        for v in range(nval):
                            # hashed: v = channel c; dense: v = roff*2 + c
                            nc.vector.tensor_tensor(
                                out=bass.AP(jf.tensor, jf.offset,
                                            [jf.ap[0], [16, ccols], [1, 16]]),
                                in0=bass.AP(gf.tensor, gf.offset + v,
                                            [gf.ap[0], [64, ccols], [2, 16]]),
                                in1=bass.AP(mf.tensor, mf.offset,
                                            [mf.ap[0], [16, ccols], [1, 16]]),
                                op=Alu.mult)
                            nc.vector.tensor_reduce(
                                out=bass.AP(feats_f.tensor,
                                            feats_f.offset + c0 * nval + v,
                                            [feats_f.ap[0], [nval, ccols]]),
                                in_=bass.AP(jf.tensor, jf.offset,
                                            [jf.ap[0], [16, ccols], [1, 16]]),
                                axis=mybir.AxisListType.X, op=Alu.add)

                    # ---- weights: w8[k] = fx_bx * fy_by * fz_bz ----
                    f2_f = f2[:, :, :, :]
                    xyw = sb.tile([P, 2, 2, cn], dt.float32, tag="xyw")
                    wx = bass.AP(f2_f.tensor, f2[:, 0, 0, :].offset,
                                 [f2_f.ap[0], [3 * cn, 2], [0, 2], [1, cn]])
                    wy = bass.AP(f2_f.tensor, f2[:, 0, 1, :].offset,
                                 [f2_f.ap[0], [0, 2], [3 * cn, 2], [1, cn]])
                    nc.vector.tensor_tensor(
                        out=xyw[:, :, :, :], in0=wx, in1=wy, op=Alu.mult)
                    w8t = sb.tile([P, 8, cn], dt.float32, tag="w8")
                    xyw_f = xyw[:, :, :, :]
                    w8_f = w8t[:, :, :]
                    for bz in range(2):
                        in_xyw = bass.AP(xyw_f.tensor, xyw_f.offset,
                                         [xyw_f.ap[0], [2 * cn, 2], [cn, 2],
                                          [1, cn]])
                        wz = bass.AP(f2_f.tensor,
                                     f2[:, bz, 2, :].offset,
                                     [f2_f.ap[0], [0, 2], [0, 2], [1, cn]])
                        o_w = bass.AP(w8_f.tensor, w8_f.offset + bz * cn,
                                      [w8_f.ap[0], [4 * cn, 2], [2 * cn, 2],
                                       [1, cn]])
                        nc.vector.tensor_tensor(out=o_w, in0=in_xyw, in1=wz,
                                                op=Alu.mult)

                    if pending is not None:
                        do_interp(pending)
                    pending = dict(li=li, dense=dense, feats=feats, w8t=w8t,
                                   outt=outt, cn=cn, jb=jb,
                                   last=(li == len(levels) - 1))

            do_interp(pending)

    nc.compile()
    return nc


_BUILD_CACHE = {}


def _get_nc(npc, nc_cols, cn_tile, levels):
    key = (npc, nc_cols, cn_tile, tuple(levels))
    if key not in _BUILD_CACHE:
        _BUILD_CACHE[key] = _build(npc, nc_cols, cn_tile, levels)
    return _BUILD_CACHE[key]


def kernel(inputs: np.ndarray, embeddings: np.ndarray, _trace=False) -> np.ndarray:
    from concourse.bass_utils import run_bass_kernel_spmd

    inputs = np.ascontiguousarray(inputs, dtype=np.float32)
    embeddings = np.ascontiguousarray(embeddings, dtype=np.float32)
    B = inputs.shape[0]

    pts_pad = np.zeros((B_PAD, 3), dtype=np.float32)
    pts_pad[:B] = inputs
    nc = _get_nc(NPC, NC_COLS, CN_TILE, list(range(L)))
    in_maps = [dict(pts=pts_pad[c * NPC:(c + 1) * NPC], emb=embeddings)
               for c in range(N_CORES)]
    import time as _time
    _t0 = _time.time()
    r = run_bass_kernel_spmd(nc, in_maps, core_ids=list(range(N_CORES)),
                             trace=_trace)
    kernel._last_result = r
    kernel._last_wall_s = _time.time() - _t0
    out = np.concatenate([r.results[c]["out"] for c in range(N_CORES)], axis=0)
    kernel._last_exec_ns = r.exec_time_ns
    return out[:B]


# revision 6
# speedup vs baseline: 1.4320x; 1.0162x over previous
"""Trainium2 Bass kernel for nn_HashEncoder (instant-NGP hash-grid encoder).

Contract: kernel(inputs, embeddings) -> [1M, 32] f32.
Sharding: data-parallel over points, 8 cores; full table in each core's HBM.

Gather strategy: the per-instruction indirect-DMA path costs ~1.15us per 128
descriptors (Pool-seq/SWDGE serial), so all corner fetches go through
InstDMAGatherAnt (dma_gather), which amortizes thousands of descriptors per
instruction and parallelizes descriptor generation across the 4 SWDGE queues.
dma_gather addresses 256B blocks via int16 indices, so a preprocessing pass
builds, per level, two copies of the level table (row-shifted by 0 and 16),
making every row reachable at in-block position idx&15 of block
(idx>>5) + ((idx>>4)&1)*nblk  (max 2*16384 = 2^15 blocks, exactly int16).
The 8B row is extracted from each gathered 256B block on DVE via a one-hot
mask (is_equal vs iota16) + multiply + segmented reduce; dense levels extract
the two consecutive rows (x, x+1) from the same block with the same mask.
"""
import sys

if "/opt/trn_rl_repo" not in sys.path:
    sys.path.insert(0, "/opt/trn_rl_repo")

import numpy as np

# ---- problem constants (hardcoded per harness contract) ----
D, L, C, H = 3, 16, 2, 16
T = 2 ** 19
BOUND = 1.0
PRIMES = (1, 2654435761, 805459861)
B_FULL = 1_000_000
N_CORES = 8


def _make_offsets():
    offs, o = [0], 0
    for l in range(L):
        res = H * (2 ** l)
        o += min(T, (res + 1) ** D)
        offs.append(o)
    return offs


OFFSETS = _make_offsets()
N_PARAMS = OFFSETS[-1]  # 7131219

# per-core point layout: NPC points = 128 partitions x NC cols, point(p, j) = p*NC + j
NC_COLS = 977
NPC = 128 * NC_COLS          # 125056
B_PAD = NPC * N_CORES        # 1000448
CN_TILE = 96                 # cols per SBUF tile
MASK19 = 0x7FFFF
NIDX = 4096                  # gather requests per dma_gather instruction
GCOLS = NIDX // 128          # 48 gathered blocks per partition per instr

# per-level block-table geometry (2 shifted copies of 32-row blocks)
def _lvl_geom():
    g = []
    base = 0  # in 32-row blocks within tbl2
    for l in range(L):
        size = OFFSETS[l + 1] - OFFSETS[l]
        nblk = -(-size // 32)
        nblk += nblk & 1          # even so copies stay 128-f32 aligned
        g.append((size, nblk, base))
        base += 2 * nblk
    return g, base


LGEOM, TBL2_BLOCKS = _lvl_geom()


def _build(npc, nc_cols, cn_tile, levels):
    import concourse.bass as bass
    import concourse.tile as tile
    from concourse import bacc, mybir

    dt = mybir.dt
    Alu = mybir.AluOpType
    P = 128

    nc = bacc.Bacc("TRN2", target_bir_lowering=False, debug=False,
                   enable_asserts=False, num_devices=N_CORES,
                   num_swdge_queues=4)
    pts_d = nc.dram_tensor("pts", [npc, 3], dt.float32, kind="ExternalInput")
    emb_d = nc.dram_tensor("emb", [N_PARAMS, C], dt.float32, kind="ExternalInput")
    nout = 2 * len(levels)
    out_d = nc.dram_tensor("out", [npc, nout], dt.float32, kind="ExternalOutput")
    # block table: per level 2 shifted copies, 32 rows (64 f32) per block
    # (+8 blocks of slack for zero-fill overshoot)
    tbl2_d = nc.dram_tensor("tbl2", [(TBL2_BLOCKS + 8) * 32, 2], dt.float32,
                            kind="Internal")
    # int16 scratch for the idx partition-fold roundtrip (4 rotating slots)
    scr_d = nc.dram_tensor("scri", [4 * 8 * cn_tile * 128], dt.int16,
                           kind="Internal")

    pts_v = pts_d.ap().rearrange("(p n) d -> p n d", p=P)   # [128, nc_cols, 3]
    out_v = out_d.ap().rearrange("(p n) c -> p n c", p=P)   # [128, nc_cols, nout]
    emb_flat = emb_d.ap().rearrange("(p n) c -> p (n c)", p=1)  # [1, N*2] view
    tbl2_flat = tbl2_d.ap().rearrange("(p n) c -> p (n c)", p=1)
    scr_v = scr_d.ap()

    col_tiles = []
    jb = 0
    while jb < nc_cols:
        cn = min(cn_tile, nc_cols - jb)
        col_tiles.append((jb, cn))
        jb += cn

    _qctr = [0]

    def next_q():
        q = _qctr[0] & 3
        _qctr[0] += 1
        return q

    with tile.TileContext(nc) as tc:
        with tc.tile_pool(name="sb", bufs=2) as sb, \
             tc.tile_pool(name="sb1", bufs=1) as sb1, \
             tc.tile_pool(name="consts", bufs=1) as cpool:

            _consts = {}

            def cu(val):
                if val not in _consts:
                    t = cpool.tile([P, 1], dt.uint32, tag=f"c{val}")
                    nc.vector.memset(t[:, :], val)
                    _consts[val] = t
                return _consts[val][:, :1]

            def ibc(val, shape_free):
                return cu(val).to_broadcast([P] + shape_free)

            # ---- preprocessing: build tbl2 (2 shifted copies per level) ----
            # bounce through SBUF in [128, 4096] f32 (2MB) chunks
            CHW = 1024
            zt = cpool.tile([P, 512], dt.float32, tag="zt")
            nc.vector.memset(zt[:, :], 0.0)
            for l in range(L):
                size, nblk, base = LGEOM[l]
                for cpy in range(2):
                    src0 = (OFFSETS[l] + 16 * cpy) * 2        # f32 offset
                    dst0 = (base + cpy * nblk) * 64
                    want = nblk * 32 * 2                       # f32 to fill
                    avail = max(0, min(want, N_PARAMS * 2 - src0))
                    o = 0
                    while o + P <= avail:
                        w = min(CHW, (avail - o) // P)
                        bt = sb.tile([P, CHW], dt.float32, tag="ppb")
                        nc.sync.dma_start(
                            out=bt[:, :w],
                            in_=bass.AP(emb_flat.tensor, src0 + o,
                                        [[w, P], [1, w]]))
                        nc.sync.dma_start(
                            out=bass.AP(tbl2_flat.tensor, dst0 + o,
                                        [[w, P], [1, w]]),
                            in_=bt[:, :w])
                        o += P * w
                    rem = avail - o
                    if rem > 0:          # <128 f32 tail of real data
                        bt = sb.tile([P, CHW], dt.float32, tag="ppb")
                        nc.sync.dma_start(
                            out=bt[0:1, :rem],
                            in_=bass.AP(emb_flat.tensor, src0 + o,
                                        [[rem, 1], [1, rem]]))
                        nc.sync.dma_start(
                            out=bass.AP(tbl2_flat.tensor, dst0 + o,
                                        [[rem, 1], [1, rem]]),
                            in_=bt[0:1, :rem])
                        o += rem
                    # zero the remainder [o, want) (NaN safety; may overshoot
                    # by <512 f32 into the next region, written later / slack)
                    z = want - o
                    if z > 0:
                        zr = min(P, z)
                        nw = -(-z // zr)
                        nc.sync.dma_start(
                            out=bass.AP(tbl2_flat.tensor, dst0 + o,
                                        [[nw, zr], [1, nw]]),
                            in_=zt[:zr, :nw])

            # iota16 f32 constant [P, 16]
            iota16 = cpool.tile([P, 16], dt.float32, tag="iota16")
            iotau = cpool.tile([P, 16], dt.uint32, tag="iotau")
            nc.gpsimd.iota(iotau[:, :], pattern=[[1, 16]], base=0,
                           channel_multiplier=0)
            nc.vector.tensor_copy(out=iota16[:, :], in_=iotau[:, :])

            def do_interp(pend):
                # trilinear interp: per channel prod = w * feats, segmented
                # reduce over corners (identical memory layout to v1)
                li = pend["li"]
                dense = pend["dense"]
                feats_t = pend["feats"]
                w8t = pend["w8t"]
                outt_t = pend["outt"]
                cn = pend["cn"]
                outt_f = outt_t[:, :, :]
                w8_f = w8t[:, :, :]
                feats_f = feats_t[:, :, :]
                for c in range(2):
                    prod = sb.tile([P, cn, 8], dt.float32, tag="prod")
                    prod_f = prod[:, :, :]
                    if dense:
                        for bx in range(2):
                            w_v = bass.AP(w8_f.tensor,
                                          w8_f.offset + bx * 4 * cn,
                                          [w8_f.ap[0], [1, cn], [cn, 4]])
                            f_v = bass.AP(feats_f.tensor,
                                          feats_f.offset + bx * 2 + c,
                                          [feats_f.ap[0], [4, cn], [4 * cn, 4]])
                            o_v = bass.AP(prod_f.tensor,
                                          prod_f.offset + bx * 4,
                                          [prod_f.ap[0], [8, cn], [1, 4]])
                            nc.vector.tensor_tensor(
                                out=o_v, in0=w_v, in1=f_v, op=Alu.mult)
                    else:
                        w_v = bass.AP(w8_f.tensor, w8_f.offset,
                                      [w8_f.ap[0], [1, cn], [cn, 8]])
                        f_v = bass.AP(feats_f.tensor, feats_f.offset + c,
                                      [feats_f.ap[0], [2, cn], [2 * cn, 8]])
                        nc.vector.tensor_tensor(
                            out=prod_f, in0=w_v, in1=f_v, op=Alu.mult)
                    res_v = bass.AP(outt_f.tensor,
                                    outt_f.offset + li * 2 + c,
                                    [outt_f.ap[0], [nout, cn]])
                    nc.vector.tensor_reduce(
                        out=res_v, in_=prod_f,
                        axis=mybir.AxisListType.X, op=Alu.add)
                if pend["last"]:
                    jb = pend["jb"]
                    nc.sync.dma_start(out=out_v[:, jb:jb + cn, :],
                                      in_=outt_f)

            pending = None

            for ti, (jb, cn) in enumerate(col_tiles):
                pts_t = sb.tile([P, cn, 3], dt.float32, tag="pts")
                nc.sync.dma_start(out=pts_t[:, :, :], in_=pts_v[:, jb:jb + cn, :])

                xn = sb.tile([P, cn, 3], dt.float32, tag="xn")
                nc.vector.tensor_scalar(
                    out=xn[:, :, :], in0=pts_t[:, :, :], scalar1=0.5, scalar2=0.5,
                    op0=Alu.mult, op1=Alu.add)
                nc.vector.tensor_scalar(
                    out=xn[:, :, :], in0=xn[:, :, :], scalar1=1.0, scalar2=0.0,
                    op0=Alu.min, op1=Alu.max)

                outt = sb.tile([P, cn, nout], dt.float32, tag="outt")

                for li, l in enumerate(levels):
                    res = H * (2 ** l)
                    size, nblk, lbase = LGEOM[l]
                    dense = (res + 1) ** D <= size
                    K = 4 if dense else 8

                    pos3 = sb.tile([P, cn, 3], dt.float32, tag="pos3")
                    nc.vector.tensor_scalar(
                        out=pos3[:, :, :], in0=xn[:, :, :], scalar1=float(res),
                        scalar2=None, op0=Alu.mult)

                    pgu = sb.tile([P, cn, 3], dt.uint32, tag="pgu")
                    rf = sb.tile([P, cn, 3], dt.float32, tag="rf")
                    gt = sb.tile([P, cn, 3], dt.float32, tag="gtf")
                    nc.vector.tensor_copy(out=pgu[:, :, :], in_=pos3[:, :, :])
                    nc.vector.tensor_copy(out=rf[:, :, :], in_=pgu[:, :, :])
                    nc.vector.tensor_tensor(
                        out=gt[:, :, :], in0=rf[:, :, :], in1=pos3[:, :, :],
                        op=Alu.is_gt)
                    nc.vector.tensor_tensor(
                        out=rf[:, :, :], in0=rf[:, :, :], in1=gt[:, :, :],
                        op=Alu.subtract)
                    nc.vector.tensor_scalar(
                        out=rf[:, :, :], in0=rf[:, :, :], scalar1=float(res - 1),
                        scalar2=None, op0=Alu.min)
                    f2 = sb.tile([P, 2, 3, cn], dt.float32, tag="f2")
                    frac = sb.tile([P, cn, 3], dt.float32, tag="frac")
                    nc.vector.tensor_tensor(
                        out=frac[:, :, :], in0=pos3[:, :, :], in1=rf[:, :, :],
                        op=Alu.subtract)
                    nc.vector.tensor_copy(out=pgu[:, :, :], in_=rf[:, :, :])
                    for d in range(3):
                        nc.vector.tensor_copy(
                            out=f2[:, 1, d, :], in_=frac[:, :, d])
                        nc.vector.tensor_scalar(
                            out=f2[:, 0, d, :], in0=frac[:, :, d], scalar1=-1.0,
                            scalar2=-1.0, op0=Alu.mult, op1=Alu.subtract)

                    # ---- corner term pairs trm[d][0/1]: [P, cn] uint32 ----
                    trm = sb.tile([P, 3, 2, cn], dt.uint32, tag="trm")
                    nc.vector.tensor_copy(out=trm[:, 0, 0, :], in_=pgu[:, :, 0])
                    nc.vector.tensor_tensor(
                        out=trm[:, 0, 1, :], in0=pgu[:, :, 0], in1=ibc(1, [cn]),
                        op=Alu.add)
                    if dense:
                        s1, s2 = res + 1, (res + 1) ** 2
                        for d, s in ((1, s1), (2, s2)):
                            nc.vector.tensor_tensor(
                                out=trm[:, d, 0, :], in0=pgu[:, :, d],
                                in1=ibc(s, [cn]), op=Alu.mult)
                            nc.vector.tensor_tensor(
                                out=trm[:, d, 1, :], in0=trm[:, d, 0, :],
                                in1=ibc(s, [cn]), op=Alu.add)
                    else:
                        nbits = l + 5
                        nch = -(-nbits // 5)
                        for d in (1, 2):
                            p = PRIMES[d]
                            acc = None
                            for jc in range(nch):
                                pk = (p << (5 * jc)) & MASK19
                                nib = sb.tile([P, cn], dt.uint32, tag="nib")
                                if jc == 0:
                                    nc.vector.tensor_tensor(
                                        out=nib[:, :], in0=pgu[:, :, d],
                                        in1=ibc(31, [cn]),
                                        op=Alu.bitwise_and)
                                else:
                                    nc.vector.tensor_tensor(
                                        out=nib[:, :], in0=pgu[:, :, d],
                                        in1=ibc(5 * jc, [cn]),
                                        op=Alu.logical_shift_right)
                                    nc.vector.tensor_tensor(
                                        out=nib[:, :], in0=nib[:, :],
                                        in1=ibc(31, [cn]),
                                        op=Alu.bitwise_and)
                                nc.vector.tensor_tensor(
                                    out=nib[:, :], in0=nib[:, :],
                                    in1=ibc(pk, [cn]), op=Alu.mult)
                                nc.vector.tensor_tensor(
                                    out=nib[:, :], in0=nib[:, :],
                                    in1=ibc(MASK19, [cn]),
                                    op=Alu.bitwise_and)
                                if acc is None:
                                    acc = sb.tile([P, cn], dt.uint32, tag="hacc")
                                    nc.vector.tensor_copy(out=acc[:, :], in_=nib[:, :])
                                else:
                                    nc.vector.tensor_tensor(
                                        out=acc[:, :], in0=acc[:, :],
                                        in1=nib[:, :], op=Alu.add)
                            nc.vector.tensor_copy(out=trm[:, d, 0, :], in_=acc[:, :])
                            nc.vector.tensor_tensor(
                                out=trm[:, d, 1, :], in0=trm[:, d, 0, :],
                                in1=ibc(p & MASK19, [cn]), op=Alu.add)

                    # ---- combine to K corner indices (k = bx*4 + by*2 + bz) ----
                    comb_op = Alu.add if dense else Alu.bitwise_xor
                    trm_f = trm[:, :, :, :]
                    part = trm_f.ap[0]
                    xy = sb.tile([P, 2, 2, cn], dt.uint32, tag="xy")
                    in_x = bass.AP(trm_f.tensor, trm[:, 0, 0, :].offset,
                                   [part, [cn, 2], [0, 2], [1, cn]])
                    in_y = bass.AP(trm_f.tensor, trm[:, 1, 0, :].offset,
                                   [part, [0, 2], [cn, 2], [1, cn]])
                    nc.vector.tensor_tensor(
                        out=xy[:, :, :, :], in0=in_x, in1=in_y, op=comb_op)
                    idx8 = sb.tile([P, 8, cn], dt.uint32, tag="idx8")
                    xy_f = xy[:, :, :, :]
                    idx8_f = idx8[:, :, :]
                    for bz in range(2):
                        in_xy = bass.AP(xy_f.tensor, xy_f.offset,
                                        [xy_f.ap[0], [2 * cn, 2], [cn, 2],
                                         [1, cn]])
                        in_z = bass.AP(trm_f.tensor,
                                       trm[:, 2, bz, :].offset,
                                       [part, [0, 2], [0, 2], [1, cn]])
                        o_z = bass.AP(idx8_f.tensor, idx8_f.offset + bz * cn,
                                      [idx8_f.ap[0], [4 * cn, 2], [2 * cn, 2],
                                       [1, cn]])
                        nc.vector.tensor_tensor(
                            out=o_z, in0=in_xy, in1=in_z, op=comb_op)
                    if not dense:
                        nc.vector.tensor_tensor(
                            out=idx8[:, :, :], in0=idx8[:, :, :],
                            in1=ibc(MASK19, [8, cn]),
                            op=Alu.bitwise_and)

                    # ---- block index + in-block position (level-relative) ----
                    kcn = K * cn
                    idxK = bass.AP(idx8_f.tensor, idx8_f.offset,
                                   [idx8_f.ap[0], [1, kcn]])
                    bi = sb.tile([P, kcn], dt.uint32, tag="bi")
                    tmp = sb.tile([P, kcn], dt.uint32, tag="btmp")
                    nc.vector.tensor_tensor(
                        out=bi[:, :kcn], in0=idxK, in1=ibc(5, [kcn]),
                        op=Alu.logical_shift_right)
                    nc.vector.tensor_tensor(
                        out=tmp[:, :kcn], in0=idxK, in1=ibc(4, [kcn]),
                        op=Alu.logical_shift_right)
                    nc.vector.tensor_tensor(
                        out=tmp[:, :kcn], in0=tmp[:, :kcn], in1=ibc(1, [kcn]),
                        op=Alu.bitwise_and)
                    nc.vector.tensor_tensor(
                        out=tmp[:, :kcn], in0=tmp[:, :kcn],
                        in1=ibc(nblk, [kcn]), op=Alu.mult)
                    nc.vector.tensor_tensor(
                        out=bi[:, :kcn], in0=bi[:, :kcn], in1=tmp[:, :kcn],
                        op=Alu.add)
                    posf = sb.tile([P, kcn], dt.float32, tag="posf")
                    nc.vector.tensor_tensor(
                        out=tmp[:, :kcn], in0=idxK, in1=ibc(15, [kcn]),
                        op=Alu.bitwise_and)
                    nc.vector.tensor_copy(out=posf[:, :kcn], in_=tmp[:, :kcn])
                    bi16 = sb.tile([P, kcn], dt.int16, tag="bi16")
                    nc.vector.tensor_copy(out=bi16[:, :kcn], in_=bi[:, :kcn])

                    # ---- wrap to dma_gather idx layout ----
                    # request i = c*128 + (16g+q) needs its block index at
                    # idxs[q, 8c+g]. Fold partitions via a DRAM roundtrip,
                    # interleave (g,c)->(c,g) on DVE, replicate to all groups.
                    scr_s = (li % 4) * (8 * cn * P)      # int16 slot
                    nc.sync.dma_start(
                        out=bass.AP(scr_v.tensor, scr_s,
                                    [[kcn, P], [1, kcn]]),
                        in_=bi16[:, :kcn])
                    nreq = kcn * P
                    nchunk = -(-nreq // NIDX)
                    ncols16 = nchunk * (NIDX // 16)
                    wrapT = sb1.tile([P, 8 * kcn], dt.int16, tag="wrapT")
                    # wrapT[q, g*kcn + c] = scr[(16g+q)*kcn + c]
                    nc.sync.dma_start(
                        out=wrapT[0:16, :],
                        in_=bass.AP(scr_v.tensor, scr_s,
                                    [[kcn, 16], [16 * kcn, 8], [1, kcn]]))
                    idxw = sb.tile([P, ncols16], dt.int16, tag="idxw")
                    if ncols16 > 8 * kcn:
                        nc.vector.memset(idxw[:, :], 0)
                    wf = wrapT[:, :]
                    of = idxw[:, :]
                    nc.vector.tensor_copy(
                        out=bass.AP(of.tensor, of.offset,
                                    [[of.ap[0][0], 16], [8, kcn], [1, 8]]),
                        in_=bass.AP(wf.tensor, wf.offset,
                                    [[wf.ap[0][0], 16], [1, kcn], [kcn, 8]]))
                    # replicate partitions 0:16 -> all 8 groups
                    for g in range(1, 8):
                        nc.sync.dma_start(
                            out=idxw[16 * g:16 * (g + 1), :],
                            in_=idxw[0:16, :])

                    # ---- gathers: dma_gather per chunk of NIDX requests ----
                    in_ap = bass.AP(tbl2_flat.tensor, lbase * 64,
                                    [[64, 2 * nblk], [1, 64]])
                    feats = sb.tile([P, kcn, 2 if not dense else 4],
                                    dt.float32,
                                    tag="featsd" if dense else "feats")
                    gts = []
                    for ci in range(nchunk):
                        gtile = sb.tile([P, GCOLS, 64], dt.float32,
                                        tag=f"gt{ci % 2}")
                        nc.gpsimd.dma_gather(
                            out_ap=gtile[:, :, :], in_ap=in_ap,
                            idxs_ap=idxw[:, ci * (NIDX // 16):
                                         (ci + 1) * (NIDX // 16)],
                            num_idxs=NIDX, num_idxs_reg=NIDX,
                            elem_size=64, queue_num=next_q(),
                            single_packet=False)
                        gts.append((ci, gtile))

                    # ---- extraction: one-hot over 16 rows ----
                    nval = 2 if not dense else 4
                    feats_f = feats[:, :, :]
                    for ci, gtile in gts:
                        c0 = ci * GCOLS            # corner-col base
                        ccols = min(GCOLS, kcn - c0)
                        if ccols <= 0:
                            break
                        gf = gtile[:, :, :]
                        mask = sb.tile([P, GCOLS, 16], dt.float32, tag="mask")
                        mf = mask[:, :, :]
                        pf = posf[:, :]
                        nc.vector.tensor_tensor(
                            out=bass.AP(mf.tensor, mf.offset,
                                        [mf.ap[0], [16, ccols], [1, 16]]),
                            in0=bass.AP(iota16[:, :].tensor,
                                        iota16[:, :].offset,
                                        [iota16[:, :].ap[0], [0, ccols],
                                         [1, 16]]),
                            in1=bass.AP(pf.tensor, pf.offset + c0,
                                        [pf.ap[0], [1, ccols], [0, 16]]),
                            op=Alu.is_equal)
                        junk = sb.tile([P, GCOLS, 16], dt.float32, tag="junk")
                        jf = junk[:, :, :]
                        for v in range(nval):
                            # hashed: v = channel c; dense: v = roff*2 + c
                            nc.vector.tensor_tensor(
                                out=bass.AP(jf.tensor, jf.offset,
                                            [jf.ap[0], [16, ccols], [1, 16]]),
                                in0=bass.AP(gf.tensor, gf.offset + v,
                                            [gf.ap[0], [64, ccols], [2, 16]]),
                                in1=bass.AP(mf.tensor, mf.offset,
                                            [mf.ap[0], [16, ccols], [1, 16]]),
                                op=Alu.mult)
                            nc.vector.tensor_reduce(
                                out=bass.AP(feats_f.tensor,
                                            feats_f.offset + c0 * nval + v,
                                            [feats_f.ap[0], [nval, ccols]]),
                                in_=bass.AP(jf.tensor, jf.offset,
                                            [jf.ap[0], [16, ccols], [1, 16]]),
                                axis=mybir.AxisListType.X, op=Alu.add)

                    # ---- weights: w8[k] = fx_bx * fy_by * fz_bz ----
                    f2_f = f2[:, :, :, :]
                    xyw = sb.tile([P, 2, 2, cn], dt.float32, tag="xyw")
                    wx = bass.AP(f2_f.tensor, f2[:, 0, 0, :].offset,
                                 [f2_f.ap[0], [3 * cn, 2], [0, 2], [1, cn]])
                    wy = bass.AP(f2_f.tensor, f2[:, 0, 1, :].offset,
                                 [f2_f.ap[0], [0, 2], [3 * cn, 2], [1, cn]])
                    nc.vector.tensor_tensor(
                        out=xyw[:, :, :, :], in0=wx, in1=wy, op=Alu.mult)
                    w8t = sb.tile([P, 8, cn], dt.float32, tag="w8")
                    xyw_f = xyw[:, :, :, :]
                    w8_f = w8t[:, :, :]
                    for bz in range(2):
                        in_xyw = bass.AP(xyw_f.tensor, xyw_f.offset,
                                         [xyw_f.ap[0], [2 * cn, 2], [cn, 2],
                                          [1, cn]])
                        wz = bass.AP(f2_f.tensor,
                                     f2[:, bz, 2, :].offset,
                                     [f2_f.ap[0], [0, 2], [0, 2], [1, cn]])
                        o_w = bass.AP(w8_f.tensor, w8_f.offset + bz * cn,
                                      [w8_f.ap[0], [4 * cn, 2], [2 * cn, 2],
                                       [1, cn]])
                        nc.vector.tensor_tensor(out=o_w, in0=in_xyw, in1=wz,
                                                op=Alu.mult)

                    if pending is not None:
                        do_interp(pending)
                    pending = dict(li=li, dense=dense, feats=feats, w8t=w8t,
                                   outt=outt, cn=cn, jb=jb,
                                   last=(li == len(levels) - 1))

            do_interp(pending)

    nc.compile()
    return nc


_BUILD_CACHE = {}


def _get_nc(npc, nc_cols, cn_tile, levels):
    key = (npc, nc_cols, cn_tile, tuple(levels))
    if key not in _BUILD_CACHE:
        _BUILD_CACHE[key] = _build(npc, nc_cols, cn_tile, levels)
    return _BUILD_CACHE[key]


def kernel(inputs: np.ndarray, embeddings: np.ndarray, _trace=False) -> np.ndarray:
    from concourse.bass_utils import run_bass_kernel_spmd

    inputs = np.ascontiguousarray(inputs, dtype=np.float32)
    embeddings = np.ascontiguousarray(embeddings, dtype=np.float32)
    B = inputs.shape[0]

    pts_pad = np.zeros((B_PAD, 3), dtype=np.float32)
    pts_pad[:B] = inputs
    nc = _get_nc(NPC, NC_COLS, CN_TILE, list(range(L)))
    in_maps = [dict(pts=pts_pad[c * NPC:(c + 1) * NPC], emb=embeddings)
               for c in range(N_CORES)]
    import time as _time
    _t0 = _time.time()
    r = run_bass_kernel_spmd(nc, in_maps, core_ids=list(range(N_CORES)),
                             trace=_trace)
    kernel._last_result = r
    kernel._last_wall_s = _time.time() - _t0
    out = np.concatenate([r.results[c]["out"] for c in range(N_CORES)], axis=0)
    kernel._last_exec_ns = r.exec_time_ns
    return out[:B]
